# revision 5
# baseline (speedup 1.0000x reference)
"""Butterfly rotation (10 stages, DIM=1024) on 8 Trainium2 NeuronCores — fp16.

Math: the whole transform is linear.  Stages 0..8 (strides 1..256) mix only
within 512-wide halves, so their composite is block-diagonal with two dense
512x512 blocks (precomputed on host from `angles`, cast to fp16).  Stage 9
(stride 512) pairs 128-dim chunks (cg, cg+4) and is applied on-chip as a
per-element rotation with per-partition cos/sin scalars.

Device layout (per core, rows sharded 8192/core; pure data parallelism):
  - host pre-transposes each core's shard to dim-major fp16 tiles
    xin[g, h, p, lc*512 + r] = x[g*512 + r, (4h + lc)*128 + p]
    (g: 16 row-groups, h: dim half, lc: chunk-in-half, p: partition, r: row)
  - each group splits into two independent chunk quads {q, q+2, q+4, q+6}
    (q = 0, 1): stage 8 couples (c, c+2), stage 9 couples (c, c+4).
  - B-quads (3 of 4): 16 fp16 matmuls [K=128, M=128, N=512] (4 accumulating
    per output chunk, stages 0..8 folded into the weights) -> PSUM; stage 9
    evicts PSUM -> SBUF fp16 (ACT cross-term pre-scale + DVE fused
    scalar_tensor_tensor).  fp16 matmuls stream 1 cycle/row, 4x faster than
    fp32, so the PE is no longer the baseline's ~269 us bottleneck.
  - A-quads (1 of 4, evenly spread): only stages 0..7 on the PE (8 matmuls),
    stages 8 AND 9 on ACT/DVE.  The 3:1 mix balances PE against ACT/DVE so
    all engines land near the HBM roofline (~33.5 MB/core fp16); the
    fraction is a sharp optimum (measured: f=0.19 -> +13 us, f=0.31 -> +32).
  - issue order is pair-major so each stage-9 pair's PSUM banks drain
    (ACT/DVE) while the PE runs the next pair's matmuls.
  - ALL DMA (loads and stores) issues from the idle Sync/SP HWDGE ring:
    store dma_starts on the ACT ring measurably steal ACT sequencer time
    (moving them to sync was worth ~15 us/rep; scalar-ring 4-way split
    stores were ~35 us/rep worse).
  - host inverse-permutes + upcasts the fp16 output tiles.
"""

import os
import sys

sys.path.insert(0, "/opt/trn_rl_repo")

os.environ["BASS_NEVER_TRACE"] = "1"

import numpy as np

DIM = 1024
STAGES = 10
N_CORES = 8
ROWS_PER_CORE = 8192
GROUP_ROWS = 512
N_GROUPS = ROWS_PER_CORE // GROUP_ROWS  # 16


def _stage_idx(dim, stage):
    stride = 2**stage
    idx_i = np.arange(dim).reshape(-1, 2 * stride)[:, :stride].ravel()
    idx_j = idx_i + stride
    return idx_i, idx_j


def _butterfly_apply(v, angles, stages):
    """Apply butterfly stages to rows of v (float64, in place) and return v."""
    for s in stages:
        idx_i, idx_j = _stage_idx(v.shape[1], s)
        c = np.cos(angles[s].astype(np.float64))
        sn = np.sin(angles[s].astype(np.float64))
        vi = v[:, idx_i].copy()
        vj = v[:, idx_j].copy()
        v[:, idx_i] = c * vi - sn * vj
        v[:, idx_j] = sn * vi + c * vj
    return v


def _host_tables(angles):
    """Stages 0..8 (strides 1..256) mix only within 512-wide halves: their
    composite is block-diagonal with two dense 512x512 blocks.  Stage 9 is
    applied on-chip as a per-element rotation between chunk pairs (cg, cg+4).

    wts[k, b, m], b = c*4 + t: lhsT block for output chunk c (0..7), input
    chunk ci = 4*(c//4) + t.  mb9 = M9^T (host butterfly on identity rows),
    so lhsT[k, m] = M9[c*128+m, ci*128+k] = mb9[ci*128+k, c*128+m].

    wts7[k, b, m], b = c*2 + t (stages 0..7 only, 256-block-diag): lhsT for
    output chunk c, input chunk ci = 2*(c//2) + t — used by ACT/DVE-heavy
    groups that apply stage 8 on-chip to offload the PE.

    trig[m, :]: cols 0:4 cos9[cg], 4:8 sin9[cg], 8:12 -sin9[cg] (stage-9
    pair cg, angles th9[cg*128:+128]); cols 12:16 cos8[pidx], 16:20
    sin8[pidx], 20:24 -sin8[pidx] (stage-8 pair pidx = (0,2),(1,3),(4,6),
    (5,7), angle offsets 0,128,256,384).
    """
    mb9 = _butterfly_apply(np.eye(DIM, dtype=np.float64), angles, range(STAGES - 1))
    # off-block-diagonal must vanish for stages 0..8
    mask = np.ones((DIM, DIM), dtype=bool)
    for q in range(2):
        mask[q * 512 : (q + 1) * 512, q * 512 : (q + 1) * 512] = False
    assert abs(mb9[mask]).max() == 0.0

    wts = np.empty((128, 32, 128), dtype=np.float16)
    for c in range(8):
        for t in range(4):
            ci = 4 * (c // 4) + t
            wts[:, c * 4 + t, :] = mb9[
                ci * 128 : (ci + 1) * 128, c * 128 : (c + 1) * 128
            ].astype(np.float16)

    mb7 = _butterfly_apply(np.eye(DIM, dtype=np.float64), angles, range(STAGES - 2))
    mask7 = np.ones((DIM, DIM), dtype=bool)
    for q in range(4):
        mask7[q * 256 : (q + 1) * 256, q * 256 : (q + 1) * 256] = False
    assert abs(mb7[mask7]).max() == 0.0
    wts7 = np.empty((128, 16, 128), dtype=np.float16)
    for c in range(8):
        for t in range(2):
            ci = 2 * (c // 2) + t
            wts7[:, c * 2 + t, :] = mb7[
                ci * 128 : (ci + 1) * 128, c * 128 : (c + 1) * 128
            ].astype(np.float16)

    th9 = angles[9].astype(np.float64)
    th8 = angles[8].astype(np.float64)
    trig = np.empty((128, 24), dtype=np.float32)
    for cg in range(4):
        sl = slice(cg * 128, (cg + 1) * 128)
        trig[:, cg] = np.cos(th9[sl])
        trig[:, 4 + cg] = np.sin(th9[sl])
        trig[:, 8 + cg] = -np.sin(th9[sl])
    for pidx in range(4):
        sl = slice(pidx * 128, (pidx + 1) * 128)
        trig[:, 12 + pidx] = np.cos(th8[sl])
        trig[:, 16 + pidx] = np.sin(th8[sl])
        trig[:, 20 + pidx] = -np.sin(th8[sl])
    return wts, wts7, trig


def _pack_x(x_core, n_groups=N_GROUPS):
    # [G*512, 1024] -> [G, 2, 128, 2048] fp16 with
    # xin[g, h, p, lc*512 + r] = x[g*512 + r, (4h + lc)*128 + p]
    g = x_core.reshape(n_groups, GROUP_ROWS, 2, 4, 128)  # [g, r, h, lc, p]
    return np.ascontiguousarray(
        g.transpose(0, 2, 4, 3, 1).reshape(n_groups, 2, 128, 2048)
    ).astype(np.float16)


def _unpack_y(y_packed, n_groups=N_GROUPS):
    # quad-major: yout[g, u, p, v*1024 + is_b*512 + r] = pair cg = u + 2v, chunk
    # cg + 4*is_b:  y[g*512+r, (4*is_b + 2v + u)*128 + p]
    g = y_packed.reshape(n_groups, 2, 128, 2, 2, GROUP_ROWS)  # [g, u, p, v, b, r]
    return np.ascontiguousarray(
        g.transpose(0, 5, 4, 3, 1, 2).reshape(n_groups * GROUP_ROWS, DIM)
    ).astype(np.float32)


def _patch_tile_drain():
    """Workaround: this walrus build cannot encode semaphore waits on a
    sequencer Drain/NoOp with >1 wait ("Too many sync wait commands").
    Re-emit the TileContext tail waits as one nop per semaphore."""
    from concourse import mybir, tile
    from concourse.vector_clock import ScopedClock

    if getattr(tile.TileContext, "_drain_patched", False):
        return

    def _drain_and_barrier(self, tick_clock, wait_clock):
        nop_inst = self.nc.sync.nop(nofuse=True)
        wait_clock.add_sem_waits(
            nop_inst.ins, ScopedClock({None: tick_clock.global_clock})
        )
        si = nop_inst.ins.sync_info
        if si is not None and si.on_wait and len(si.on_wait) > 1:
            extra = si.on_wait[1:]
            si.on_wait = si.on_wait[:1]
            for w in extra:
                extra_nop = self.nc.sync.nop(nofuse=True)
                esi = extra_nop.ins.sync_info
                if esi is None:
                    extra_nop.ins.sync_info = mybir.SyncInfo(on_wait=[w], on_update=[])
                else:
                    esi.on_wait = list(esi.on_wait or []) + [w]
        self.nc.sync.drain()
        self.nc.all_engine_barrier()
        assert self.sems is not None
        popped = self.nc._tile_sem_poison_stack.pop()
        assert popped is self._sem_poison
        self.nc.clear_and_free_semaphores(list(self.sems.allocated().values()))
        self.nc.all_engine_barrier()

    tile.TileContext._drain_and_barrier = _drain_and_barrier
    tile.TileContext._drain_patched = True


def _split_multi_waits(nc, limit=1):
    """This walrus build encodes at most `limit` semaphore wait(s) per
    instruction ("Too many sync wait commands").  Hoist excess waits onto
    same-engine NoOps inserted immediately before the instruction."""
    from concourse import mybir

    counter = [0]

    def fresh_nop(engine, waits):
        counter[0] += 1
        nop = mybir.InstNoOp(
            name=f"waitsplit-{counter[0]}",
            engine=engine,
            ins=[],
            outs=[],
            bass_nofuse=True,
            sync_info=mybir.SyncInfo(on_wait=list(waits), on_update=[]),
        )
        nc.register_instruction(nop, overwrite=True)
        return nop

    for fn in nc.m.functions:
        for bb in fn.blocks:
            changed = False
            new = []
            for inst in bb.instructions:
                si = getattr(inst, "sync_info", None)
                if si is not None and si.on_wait and len(si.on_wait) > limit:
                    extra = si.on_wait[: len(si.on_wait) - limit]
                    si.on_wait = si.on_wait[len(si.on_wait) - limit :]
                    for k in range(0, len(extra), limit):
                        new.append(fresh_nop(inst.engine, extra[k : k + limit]))
                    changed = True
                new.append(inst)
            if changed:
                bb.instructions = new


# quads applying stage 8 on ACT/DVE instead of the PE (offloads the PE
# bottleneck; 8 A-quads of 32 puts PE/ACT/DVE all near the DMA roofline).
# A-quads sit on odd groups, alternating which quad, so the ACT/DVE-heavy
# bursts stay small and evenly spread.
N_A_QUADS = 8
XP_BUFS = 3
YP_BUFS = 3
TP_BUFS = 6
DMA_SPREAD = False
STORE_ON_SYNC = True
STORE_SPLIT = 2


def _is_a_quad(g, q, n_groups=N_GROUPS):
    if n_groups < 4 or not N_A_QUADS:
        return False
    qi = 2 * g + q  # quad index 0..2*n_groups-1
    step = 2 * n_groups / N_A_QUADS
    return qi in {int((i + 0.5) * step) for i in range(N_A_QUADS)}


def build_bass(n_groups=N_GROUPS, reps=1):
    """Build the Bass module for one core processing n_groups row-groups.
    reps>1 repeats the whole pipeline in-NEFF (for timing calibration)."""
    _patch_tile_drain()
    from concourse import bass, mybir, tile

    f32 = mybir.dt.float32
    f16 = mybir.dt.float16
    nc = bass.Bass("TRN2", target_bir_lowering=False, debug=False)
    xin = nc.dram_tensor("xin", [n_groups, 2, 128, 2048], f16, kind="ExternalInput")
    wts = nc.dram_tensor("wts", [128, 32, 128], f16, kind="ExternalInput")
    wts7 = nc.dram_tensor("wts7", [128, 16, 128], f16, kind="ExternalInput")
    trig = nc.dram_tensor("trig", [128, 24], f32, kind="ExternalInput")
    yout = nc.dram_tensor("yout", [n_groups, 2, 128, 2048], f16, kind="ExternalOutput")

    mult = mybir.AluOpType.mult
    add = mybir.AluOpType.add
    copy_fn = mybir.ActivationFunctionType.Copy

    with tile.TileContext(nc) as tc:
        with (
            tc.tile_pool(name="wp", bufs=1) as wp,
            tc.tile_pool(name="xp", bufs=XP_BUFS) as xp,
            tc.tile_pool(name="yp", bufs=YP_BUFS) as yp,
            tc.tile_pool(name="sp", bufs=2) as stp,
            tc.tile_pool(name="tp", bufs=TP_BUFS) as tp,
            tc.tile_pool(name="ps", bufs=8, space="PSUM") as psp,
        ):
            wt = wp.tile([128, 32, 128], f16)
            nc.sync.dma_start(wt[:], wts.ap()[:])
            wt7 = wp.tile([128, 16, 128], f16)
            nc.sync.dma_start(wt7[:], wts7.ap()[:])
            tg = wp.tile([128, 24], f32)
            nc.sync.dma_start(tg[:], trig.ap()[:])

            def b_quad(g, xt, yt, q):
                # stage 8 folded into weights; pairs (cg, cg+4) for cg in
                # (q, q+2); stage-9 drain overlaps the next pair's matmuls
                for v, cg in enumerate((q, q + 2)):
                    pos = 2 * q + v  # yt slot (quad-major)
                    pa = psp.tile([128, 512], f32, tag="pp")
                    for t in range(4):
                        nc.tensor.matmul(
                            pa[:],
                            wt[:, cg * 4 + t, :],
                            xt[:, t * 512 : (t + 1) * 512],
                            start=(t == 0),
                            stop=(t == 3),
                        )
                    pb = psp.tile([128, 512], f32, tag="pp")
                    for t in range(4):
                        nc.tensor.matmul(
                            pb[:],
                            wt[:, (cg + 4) * 4 + t, :],
                            xt[:, 2048 + t * 512 : 2048 + (t + 1) * 512],
                            start=(t == 0),
                            stop=(t == 3),
                        )
                    # stage 9: y_a = c9*pa - s9*pb ; y_b = s9*pa + c9*pb
                    t1 = tp.tile([128, 512], f32, tag="t")
                    nc.scalar.activation(
                        t1[:], pb[:], copy_fn, scale=tg[:, 8 + cg : 9 + cg]
                    )
                    nc.vector.scalar_tensor_tensor(
                        yt[:, pos * 1024 : pos * 1024 + 512],
                        pa[:], tg[:, cg : cg + 1], t1[:], mult, add,
                    )
                    t2 = tp.tile([128, 512], f32, tag="t")
                    nc.scalar.activation(
                        t2[:], pb[:], copy_fn, scale=tg[:, cg : cg + 1]
                    )
                    nc.vector.scalar_tensor_tensor(
                        yt[:, pos * 1024 + 512 : (pos + 1) * 1024],
                        pa[:], tg[:, 4 + cg : 5 + cg], t2[:], mult, add,
                    )

            def a_quad(g, xt, yt, q):
                # chunks (q, q+2, q+4, q+6): stages 0..7 on PE (2 matmuls per
                # chunk), stages 8 AND 9 on ACT/DVE.  st4 slots 0..3 hold the
                # stage-8 result for chunks q, q+2, q+4, q+6.
                st4 = stp.tile([128, 2048], f16)
                for hp in range(2):
                    c0 = q + 4 * hp
                    ps = []
                    for c in (c0, c0 + 2):
                        p = psp.tile([128, 512], f32, tag="pp")
                        for t in range(2):
                            ci = 2 * (c // 2) + t
                            nc.tensor.matmul(
                                p[:],
                                wt7[:, c * 2 + t, :],
                                xt[:, ci * 512 : (ci + 1) * 512],
                                start=(t == 0),
                                stop=(t == 1),
                            )
                        ps.append(p)
                    pidx = q + 2 * hp  # stage-8 pair (c0, c0+2)
                    pa, pb = ps
                    t1 = tp.tile([128, 512], f32, tag="t")
                    nc.scalar.activation(
                        t1[:], pb[:], copy_fn,
                        scale=tg[:, 20 + pidx : 21 + pidx],
                    )
                    nc.vector.scalar_tensor_tensor(
                        st4[:, (2 * hp) * 512 : (2 * hp + 1) * 512],
                        pa[:], tg[:, 12 + pidx : 13 + pidx], t1[:],
                        mult, add,
                    )
                    t2 = tp.tile([128, 512], f32, tag="t")
                    nc.scalar.activation(
                        t2[:], pb[:], copy_fn,
                        scale=tg[:, 12 + pidx : 13 + pidx],
                    )
                    nc.vector.scalar_tensor_tensor(
                        st4[:, (2 * hp + 1) * 512 : (2 * hp + 2) * 512],
                        pa[:], tg[:, 16 + pidx : 17 + pidx], t2[:],
                        mult, add,
                    )
                # stage 9 off SBUF fp16 (DVE runs 2x): pairs (cg, cg+4)
                for v, cg in enumerate((q, q + 2)):
                    pos = 2 * q + v
                    sa = st4[:, v * 512 : (v + 1) * 512]
                    sb = st4[:, (v + 2) * 512 : (v + 3) * 512]
                    t3 = tp.tile([128, 512], f16, tag="th")
                    nc.scalar.activation(
                        t3[:], sb, copy_fn, scale=tg[:, 8 + cg : 9 + cg]
                    )
                    nc.vector.scalar_tensor_tensor(
                        yt[:, pos * 1024 : pos * 1024 + 512],
                        sa, tg[:, cg : cg + 1], t3[:], mult, add,
                    )
                    t4 = tp.tile([128, 512], f16, tag="th")
                    nc.scalar.activation(
                        t4[:], sb, copy_fn, scale=tg[:, cg : cg + 1]
                    )
                    nc.vector.scalar_tensor_tensor(
                        yt[:, pos * 1024 + 512 : (pos + 1) * 1024],
                        sa, tg[:, 4 + cg : 5 + cg], t4[:], mult, add,
                    )

            for g in [g for _ in range(reps) for g in range(n_groups)]:
                xt = xp.tile([128, 4096], f16)
                ld2 = nc.gpsimd.dma_start if DMA_SPREAD else nc.sync.dma_start
                st2 = nc.gpsimd.dma_start if DMA_SPREAD else nc.scalar.dma_start
                nc.sync.dma_start(xt[:, 0:2048], xin.ap()[g][0])
                ld2(xt[:, 2048:4096], xin.ap()[g][1])
                yt = yp.tile([128, 4096], f16)
                st_eng = nc.sync.dma_start if STORE_ON_SYNC else nc.scalar.dma_start
                for q in range(2):
                    if _is_a_quad(g, q, n_groups):
                        a_quad(g, xt, yt, q)
                    else:
                        b_quad(g, xt, yt, q)
                    if STORE_SPLIT == 4:
                        for v in range(2):
                            st_eng(
                                yout.ap()[g][q][:, v * 1024 : (v + 1) * 1024],
                                yt[:, (2 * q + v) * 1024 : (2 * q + v + 1) * 1024],
                            )
                    elif q == 0:
                        st_eng(yout.ap()[g][0], yt[:, 0:2048])
                    else:
                        (st2 if not STORE_ON_SYNC else st_eng)(
                            yout.ap()[g][1], yt[:, 2048:4096]
                        )
    _split_multi_waits(nc)
    return nc


_CACHE = {}


def _get_nc(n_groups=N_GROUPS):
    if n_groups not in _CACHE:
        _CACHE[n_groups] = build_bass(n_groups)
    return _CACHE[n_groups]


def make_in_maps(x, angles):
    """Pack full inputs into per-core in_maps (list of dicts)."""
    x = np.asarray(x, dtype=np.float32)
    angles = np.asarray(angles, dtype=np.float32)
    wts, wts7, trig = _host_tables(angles)
    flat = x.reshape(-1, DIM)
    in_maps = []
    for k in range(N_CORES):
        shard = flat[k * ROWS_PER_CORE : (k + 1) * ROWS_PER_CORE]
        in_maps.append(
            {"xin": _pack_x(shard), "wts": wts, "wts7": wts7, "trig": trig}
        )
    return in_maps


def kernel(x, angles):
    from concourse.bass_utils import run_bass_kernel_spmd

    x = np.asarray(x)
    orig_shape = x.shape
    in_maps = make_in_maps(x, angles)
    nc = _get_nc()
    res = run_bass_kernel_spmd(nc, in_maps, core_ids=list(range(N_CORES)))
    parts = [_unpack_y(res.results[k]["yout"]) for k in range(N_CORES)]
    out = np.concatenate(parts, axis=0).reshape(orig_shape)
    return out.astype(np.float32)
